# revision 41
# baseline (speedup 1.0000x reference)
"""BertCRF forward (BERT-base encoder + CRF NLL) on 8 Trainium2 NeuronCores.

Strategy: data-parallel over the batch (32 examples -> 4 per core), params
replicated.  Each core runs the full 12-layer encoder on its 1024 tokens with
bf16 matmuls (fp32 accumulation), fp32 layernorm, max-free softmax with the
normalizer folded into an extra ones-column of V, then the classifier and an
exact CRF negative-log-likelihood computed in linear space with a fixed
per-step shift (cancels exactly between numerator and denominator).  The host
only shards inputs, pre-arranges weight layouts, and sums the 8 per-core
partial NLLs.
"""

import contextlib
import os

import numpy as np
import ml_dtypes

import concourse.bass as bass  # noqa: F401  (kept for users poking at the module)
import concourse.mybir as mybir
import concourse.tile as tile
from concourse import bacc
from concourse.bass import IndirectOffsetOnAxis
from concourse.bass_utils import run_bass_kernel_spmd
from concourse.masks import make_identity

# ---- problem constants (hardcoded per the task spec) ----
L, H, NH, DH, FF, V, K = 12, 768, 12, 64, 3072, 30522, 9
B, S = 32, 256
CORES = 8
BC = B // CORES          # 4 examples per core
T = BC * S               # 1024 tokens per core
P = 128
NT = T // P              # 8 token tiles
HC = H // P              # 6 hidden chunks
FC = FF // P             # 24 ff chunks
C_SHIFT = 2.35           # per-step CRF shift (cancels exactly in num-denom)

F32 = mybir.dt.float32
BF16 = mybir.dt.bfloat16
FP8 = mybir.dt.float8e4
I32 = mybir.dt.int32
AX = mybir.AxisListType
OP = mybir.AluOpType
AF = mybir.ActivationFunctionType
DR = mybir.MatmulPerfMode.DoubleRow

BF = ml_dtypes.bfloat16

# fp8 quantization scales (exact powers of two)
WS = 1024.0              # weight scale into fp8e4
XS = 32.0                # activation scale into fp8e4
DQ_WX = 2.0 ** -15       # dequant for w*x products
DQ_W = 2.0 ** -10        # dequant when only the weight was scaled


def _bf(x):
    return np.ascontiguousarray(np.asarray(x, dtype=np.float32)).astype(BF)


def _f8(x, scale):
    return np.ascontiguousarray(np.clip(
        np.asarray(x, dtype=np.float32) * scale, -240.0, 240.0)
    ).astype(ml_dtypes.float8_e4m3)


def _f32(x):
    return np.ascontiguousarray(np.asarray(x, dtype=np.float32))


# ---------------------------------------------------------------------------
# device program
# ---------------------------------------------------------------------------

def _layernorm(nc, tmp, out_bf, xf, gb, s1=None):
    """LN over the free dim of xf [P, H] f32 -> out_bf (bf16).

    s1, if given, is a [P, 1] tile already holding sum(xf) (computed for free
    via accum_out on the op that produced xf).
    """
    if s1 is None or os.environ.get("BERTCRF_OLDLN"):
        s1 = tmp.tile([P, 1], F32, tag="s1", name="s1")
        nc.vector.tensor_reduce(out=s1[:], in_=xf[:], axis=AX.X, op=OP.add)
    sq = tmp.tile([P, H], F32, tag="sq", name="sq")
    s2 = tmp.tile([P, 1], F32, tag="s2", name="s2")
    if os.environ.get("BERTCRF_OLDLN"):
        nc.scalar.activation(sq[:], xf[:], AF.Square, accum_out=s2[:])
    else:
        # sum(x^2) fused with the squaring, on DVE (sq is a sink)
        nc.vector.scalar_tensor_tensor(out=sq[:], in0=xf[:], scalar=1.0,
                                       in1=xf[:], op0=OP.mult, op1=OP.mult,
                                       accum_out=s2[:])
    m = tmp.tile([P, 1], F32, tag="m", name="m")
    nc.vector.tensor_scalar(out=m[:], in0=s1[:], scalar1=1.0 / H, scalar2=None,
                            op0=OP.mult)
    msq = tmp.tile([P, 1], F32, tag="msq", name="msq")
    nc.vector.tensor_tensor(out=msq[:], in0=m[:], in1=m[:], op=OP.mult)
    var = tmp.tile([P, 1], F32, tag="var", name="var")
    nc.vector.tensor_scalar(out=var[:], in0=s2[:], scalar1=1.0 / H,
                            scalar2=msq[:, :1], op0=OP.mult, op1=OP.subtract)
    # eps=1e-12 is below f32 resolution for var~O(1); bias=0.0 is identical
    sd = tmp.tile([P, 1], F32, tag="sd", name="sd")
    nc.scalar.activation(sd[:], var[:], AF.Sqrt, bias=0.0)
    rs = tmp.tile([P, 1], F32, tag="rs", name="rs")
    nc.vector.reciprocal(rs[:], sd[:])
    if gb is None:
        if os.environ.get("BERTCRF_OLDLN") or os.environ.get(
                "BERTCRF_NOSPLITOUT"):
            nc.vector.tensor_scalar(out=out_bf, in0=xf[:], scalar1=m[:, :1],
                                    scalar2=rs[:, :1], op0=OP.subtract,
                                    op1=OP.mult)
            return
        # out = rs*x - m*rs, one half on DVE, one half on ACT (in parallel)
        nmrs = tmp.tile([P, 1], F32, tag="nmrs", name="nmrs")
        nc.vector.tensor_scalar(out=nmrs[:], in0=m[:], scalar1=-1.0,
                                scalar2=rs[:, :1], op0=OP.mult, op1=OP.mult)
        HH = H // 2
        nc.vector.tensor_scalar(out=out_bf[:, 0:HH], in0=xf[:, 0:HH],
                                scalar1=m[:, :1], scalar2=rs[:, :1],
                                op0=OP.subtract, op1=OP.mult)
        nc.scalar.activation(out_bf[:, HH:H], xf[:, HH:H], AF.Identity,
                             bias=nmrs[:, :1], scale=rs[:, :1])
    else:
        G, Bb = gb
        t2 = tmp.tile([P, H], F32, tag="t2", name="t2")
        nc.vector.tensor_scalar(out=t2[:], in0=xf[:], scalar1=m[:, :1],
                                scalar2=rs[:, :1], op0=OP.subtract, op1=OP.mult)
        t3 = tmp.tile([P, H], F32, tag="t3", name="t3")
        nc.vector.tensor_tensor(out=t3[:], in0=t2[:], in1=G[:], op=OP.mult)
        nc.vector.tensor_tensor(out=out_bf, in0=t3[:], in1=Bb[:], op=OP.add)


def _transpose_tiles(nc, ps, xT, x_sb, ident, tiles):
    """x_sb [P, NT, H] token-major -> xT [P, HC, T] feature-major, per tile."""
    for g in tiles:
        for cg in range(2):          # chunk groups of 3
            pt_ = ps.tile([P, 1024], BF16, tag="ps", name="ptp", space="PSUM")
            for ci in range(3):
                c = cg * 3 + ci
                nc.tensor.matmul(
                    pt_[:, ci * P:(ci + 1) * P],
                    lhsT=x_sb[:, g, c * P:(c + 1) * P], rhs=ident[:],
                    start=True, stop=True, is_transpose=True)
            nc.scalar.activation(
                xT[:, cg * 3:cg * 3 + 3, g * P:(g + 1) * P],
                pt_[:, :384], AF.Identity, scale=XS)


def _bcast_row(nc, ps, tmp, dst, row_dram, ones_bf):
    """dst [P, H] bf16 = broadcast of a [1, H] bf16 dram row across partitions."""
    row = tmp.tile([1, H], BF16, tag="brow", name="brow")
    nc.sync.dma_start(row[:], row_dram)
    for fh in range(2):
        pb = ps.tile([P, 512], F32, tag="ps", name="pbc", space="PSUM")
        nc.tensor.matmul(pb[:, :384], lhsT=ones_bf[:, :P],
                         rhs=row[:, fh * 384:(fh + 1) * 384],
                         start=True, stop=True)
        nc.scalar.activation(dst[:, fh * 384:(fh + 1) * 384], pb[:, :384],
                             AF.Identity)


def _outer_f32(nc, ps, dst, ones_row, row):
    """dst [Np, W] f32 = row [1, W] broadcast across Np partitions (PE outer)."""
    npart, width = dst.shape[0], dst.shape[-1]
    for i in range((width + 511) // 512):
        w = min(512, width - i * 512)
        pb = ps.tile([P, 512], F32, tag="ps", name="pob", space="PSUM")
        nc.tensor.matmul(pb[:npart, :w], lhsT=ones_row,
                         rhs=row[:, i * 512:i * 512 + w], start=True, stop=True)
        nc.scalar.activation(dst[:, i * 512:i * 512 + w], pb[:npart, :w],
                             AF.Identity)


def build_nc(n_layers=L, mask_ones=True, zero_bias=True, unit_ln=True,
             debug=False):
    nc = bacc.Bacc("TRN2", target_bir_lowering=False, debug=False)

    dd = {}

    def din(name, shape, dtype):
        dd[name] = nc.dram_tensor(name, list(shape), dtype, kind="ExternalInput")
        return dd[name]

    def dout(name, shape, dtype):
        dd[name] = nc.dram_tensor(name, list(shape), dtype, kind="ExternalOutput")
        return dd[name]

    word = din("word", [V, H], BF16)
    ids = din("ids", [NT, P, 1], I32)
    pt = din("pt", [P, 2, H], BF16)
    wq_d = din("wq", [L, P, 3, 2, H], FP8)
    wk_d = din("wk", [L, P, 3, 2, H], FP8)
    wv_d = din("wv", [L, P, 3, 2, H], FP8)
    wo_d = din("wo", [L, P, 3, 2, H], FP8)
    w1_d = din("w1", [L, FC, P, 3, 2, P], FP8)   # [l, j, ki, c2, ko, m]
    w2_d = din("w2", [L, FC // 2, P, 2, H], FP8)  # [l, c2, ki, ko, n]
    clsw_d = din("clsw", [P, 3, 2, 16], FP8)      # K padded to 16
    clsb_d = din("clsb", [K, 1], F32)           # already shifted by -C_SHIFT
    din("blk2", [2, P], BF16)
    transm_d = din("transm", [K, K], F32)
    transcol_d = din("transcol", [81, 1], F32)
    startc_d = din("startc", [K, 1], F32)
    endc_d = din("endc", [K, 1], F32)
    i9_d = din("i9", [K, 1], F32)
    i81_d = din("i81", [81, 1], F32)
    labels_d = din("labels", [1, T], I32)
    maskrow_d = din("maskrow", [1, T], I32)
    if not mask_ones:
        maskcols_d = din("maskcols", [P, 2, BC], I32)
    if not zero_bias:
        bq_d = din("bq", [L, P, HC], F32)
        bk_d = din("bk", [L, P, HC], F32)
        b1_d = din("b1", [L, P, FC], F32)
        bvrow_d = din("bvrow", [L, 1, H], BF16)
        borow_d = din("borow", [L, 1, H], BF16)
        b2row_d = din("b2row", [L, 1, H], BF16)
    if not unit_ln:
        lng_d = din("lng", [L, 2, 1, H], BF16)
        lnb_d = din("lnb", [L, 2, 1, H], BF16)
        elng_d = din("elng", [2, 1, H], BF16)

    out_d = dout("out", [1, 4], F32)
    if debug:
        dout("dbg_x0", [P, NT, H], BF16)
        dout("dbg_x", [P, NT, H], BF16)
        dout("dbg_em", [K, T], F32)
        dout("dbg_ef", [K, BC], F32)
        dout("dbg_acc", [P, 4], F32)

    _build_body(nc, dd, n_layers, mask_ones, zero_bias, unit_ln, debug)
    nc.compile()
    return nc


def _build_body(nc, dd, n_layers, mask_ones, zero_bias, unit_ln, debug):
    (word, ids, pt, wq_d, wk_d, wv_d, wo_d, w1_d, w2_d, clsw_d, clsb_d,
     transm_d, transcol_d, startc_d, endc_d, i9_d, i81_d, labels_d,
     maskrow_d, out_d) = (
        dd["word"], dd["ids"], dd["pt"], dd["wq"], dd["wk"], dd["wv"],
        dd["wo"], dd["w1"], dd["w2"], dd["clsw"], dd["clsb"], dd["transm"],
        dd["transcol"], dd["startc"], dd["endc"], dd["i9"], dd["i81"],
        dd["labels"], dd["maskrow"], dd["out"])
    del maskrow_d  # names bound below where needed
    maskrow_d = dd["maskrow"]
    if not mask_ones:
        maskcols_d = dd["maskcols"]
    if not zero_bias:
        bq_d, bk_d, b1_d = dd["bq"], dd["bk"], dd["b1"]
        bvrow_d, borow_d, b2row_d = dd["bvrow"], dd["borow"], dd["b2row"]
    if not unit_ln:
        lng_d, lnb_d, elng_d = dd["lng"], dd["lnb"], dd["elng"]
    with tile.TileContext(nc) as tc, contextlib.ExitStack() as octx:
        cst = octx.enter_context(tc.tile_pool(name="cst", bufs=1))
        act = octx.enter_context(tc.tile_pool(name="act", bufs=1))
        ps = octx.enter_context(tc.tile_pool(name="ps", bufs=8, space="PSUM"))

        # ---- persistent activation buffers ----
        x_sb = act.tile([P, NT, H], BF16, tag="x_sb", name="x_sb")
        xT = act.tile([P, HC, T], FP8, tag="xT", name="xT")
        qT = act.tile([P, HC, T], BF16, tag="qT", name="qT")
        kT = act.tile([P, HC, T], BF16, tag="kT", name="kT")
        vS = act.tile([P, NT, NH, DH + 1], BF16, tag="vS", name="vS")
        cT = act.tile([P, HC, T], FP8, tag="cT", name="cT")
        hT = act.tile([P, FC, T // 2], FP8, tag="hT", name="hT")
        emT = act.tile([K, T], F32, tag="emT", name="emT")
        expEm = act.tile([K, T], F32, tag="expEm", name="expEm")

        # ---- constants ----
        ident = cst.tile([P, P], BF16, tag="ident", name="ident")
        make_identity(nc, ident[:])
        ones_bf = cst.tile([1, P], BF16, tag="ones_bf", name="ones_bf")
        nc.vector.memset(ones_bf[:], 1.0)
        # blkA/blkB: [1,128] indicators of the bottom/top 64 partitions; two
        # accumulating K=1 matmuls broadcast two [1,S] rows into the two
        # 64-partition halves of one PSUM tile
        blkA = cst.tile([1, P], BF16, tag="blkA", name="blkA")
        blkB = cst.tile([1, P], BF16, tag="blkB", name="blkB")
        nc.sync.dma_start(blkA[:], dd["blk2"][0:1])
        nc.sync.dma_start(blkB[:], dd["blk2"][1:2])
        ones_f = cst.tile([1, P], F32, tag="ones_f", name="ones_f")
        nc.vector.memset(ones_f[:], 1.0)
        onescol_f = cst.tile([P, 1], F32, tag="onescol_f", name="onescol_f")
        nc.vector.memset(onescol_f[:], 1.0)
        nc.vector.memset(vS[:, :, :, DH:DH + 1], 1.0)

        pt_sb = cst.tile([P, 2, H], BF16, tag="pt_sb", name="pt_sb")
        nc.sync.dma_start(pt_sb[:], pt[:])
        if not mask_ones:
            mcol = cst.tile([P, 2, BC], F32, tag="mcol", name="mcol")
        if not unit_ln:
            elnG = cst.tile([P, H], BF16, tag="elnG", name="elnG")
            elnB = cst.tile([P, H], BF16, tag="elnB", name="elnB")

        with contextlib.ExitStack() as ictx:
            wts = ictx.enter_context(tc.tile_pool(name="wts", bufs=1))
            tmp = ictx.enter_context(tc.tile_pool(name="tmp", bufs=3))

            if not unit_ln:
                _bcast_row(nc, ps, tmp, elnG, elng_d[0], ones_bf)
                _bcast_row(nc, ps, tmp, elnB, elng_d[1], ones_bf)

            # =========== embeddings ===========
            for g in range(NT):
                idx = tmp.tile([P, 1], I32, tag="idx", name="idx")
                nc.sync.dma_start(idx[:], ids[g])
                emb = tmp.tile([P, H], BF16, tag="emb", name="emb")
                nc.gpsimd.indirect_dma_start(
                    out=emb[:], out_offset=None, in_=word[:],
                    in_offset=IndirectOffsetOnAxis(ap=idx[:, :1], axis=0),
                )
                xf = tmp.tile([P, H], F32, tag="xf", name="xf")
                if os.environ.get("BERTCRF_OLDLN"):
                    nc.vector.tensor_tensor(out=xf[:], in0=emb[:],
                                            in1=pt_sb[:, g % 2, :], op=OP.add)
                    s1e = None
                else:
                    s1e = tmp.tile([P, 1], F32, tag="s1e", name="s1e")
                    nc.vector.scalar_tensor_tensor(
                        out=xf[:], in0=emb[:], scalar=0.0,
                        in1=pt_sb[:, g % 2, :], op0=OP.add, op1=OP.add,
                        accum_out=s1e[:])
                _layernorm(nc, tmp, x_sb[:, g, :], xf,
                           None if unit_ln else (elnG, elnB), s1=s1e)
            if debug:
                nc.sync.dma_start(dd["dbg_x0"][:], x_sb[:])

            if not mask_ones:
                mi = tmp.tile([P, 2, BC], I32, tag="mi", name="mi")
                nc.sync.dma_start(mi[:], maskcols_d[:])
                nc.vector.tensor_scalar(out=mcol[:], in0=mi[:], scalar1=1.0,
                                        scalar2=10000.0, op0=OP.subtract,
                                        op1=OP.mult)

            # =========== encoder layers ===========
            for l in range(n_layers):
                wq = wts.tile([P, 3, 2, H], FP8, tag="wq", name="wq")
                wk = wts.tile([P, 3, 2, H], FP8, tag="wk", name="wk")
                wv = wts.tile([P, 3, 2, H], FP8, tag="wv", name="wv")
                wo = wts.tile([P, 3, 2, H], FP8, tag="wo", name="wo")
                nc.sync.dma_start(wq[:], wq_d[l])
                nc.sync.dma_start(wk[:], wk_d[l])
                nc.sync.dma_start(wv[:], wv_d[l])
                nc.sync.dma_start(wo[:], wo_d[l])

                if not zero_bias:
                    bq_sb = wts.tile([P, HC], F32, tag="bq", name="bq")
                    bk_sb = wts.tile([P, HC], F32, tag="bk", name="bk")
                    b1_sb = wts.tile([P, FC], F32, tag="b1", name="b1")
                    nc.sync.dma_start(bq_sb[:], bq_d[l])
                    nc.sync.dma_start(bk_sb[:], bk_d[l])
                    nc.sync.dma_start(b1_sb[:], b1_d[l])
                    bvrow = wts.tile([1, H], BF16, tag="bvrow", name="bvrow")
                    borow = wts.tile([1, H], BF16, tag="borow", name="borow")
                    b2row = wts.tile([1, H], BF16, tag="b2row", name="b2row")
                    nc.sync.dma_start(bvrow[:], bvrow_d[l])
                    nc.sync.dma_start(borow[:], borow_d[l])
                    nc.sync.dma_start(b2row[:], b2row_d[l])
                if not unit_ln:
                    G1 = wts.tile([P, H], BF16, tag="G1", name="G1")
                    B1t = wts.tile([P, H], BF16, tag="B1t", name="B1t")
                    G2 = wts.tile([P, H], BF16, tag="G2", name="G2")
                    B2t = wts.tile([P, H], BF16, tag="B2t", name="B2t")
                    _bcast_row(nc, ps, tmp, G1, lng_d[l, 0], ones_bf)
                    _bcast_row(nc, ps, tmp, B1t, lnb_d[l, 0], ones_bf)
                    _bcast_row(nc, ps, tmp, G2, lng_d[l, 1], ones_bf)
                    _bcast_row(nc, ps, tmp, B2t, lnb_d[l, 1], ones_bf)

                # ---- transpose x -> xT ----
                _transpose_tiles(nc, ps, xT, x_sb, ident, range(NT))

                # ---- qT, kT projections (feature-major) ----
                for wmat, bname, dst in ((wq, "bq", qT), (wk, "bk", kT)):
                    for f in range(HC):
                        for t2 in range(2):
                            pm = ps.tile([P, 512], F32, tag="ps", name="pqk",
                                         space="PSUM")
                            for c2 in range(3):
                                nc.tensor.matmul(
                                    pm[:],
                                    lhsT=wmat[:, c2, :, f * P:(f + 1) * P],
                                    rhs=xT[:, 2 * c2:2 * c2 + 2,
                                           t2 * 512:(t2 + 1) * 512],
                                    start=(c2 == 0), stop=(c2 == 2),
                                    perf_mode=DR)
                            if zero_bias:
                                nc.vector.tensor_scalar(
                                    out=dst[:, f, t2 * 512:(t2 + 1) * 512],
                                    in0=pm[:], scalar1=DQ_WX, scalar2=None,
                                    op0=OP.mult)
                            else:
                                bias = (bq_sb if bname == "bq"
                                        else bk_sb)[:, f:f + 1]
                                nc.scalar.activation(
                                    dst[:, f, t2 * 512:(t2 + 1) * 512], pm[:],
                                    AF.Identity, bias=bias, scale=DQ_WX)

                # ---- V projection (token-major into vS) ----
                for g in range(NT):
                    for fh in range(2):
                        pm = ps.tile([P, 512], F32, tag="ps", name="pv",
                                     space="PSUM")
                        for c2 in range(3):
                            nc.tensor.matmul(
                                pm[:, :384],
                                lhsT=xT[:, 2 * c2:2 * c2 + 2,
                                        g * P:(g + 1) * P],
                                rhs=wv[:, c2, :, fh * 384:(fh + 1) * 384],
                                start=(c2 == 0),
                                stop=(c2 == 2 and zero_bias),
                                perf_mode=DR)
                        if not zero_bias:
                            nc.tensor.matmul(
                                pm[:, :384], lhsT=ones_bf[:, :P],
                                rhs=bvrow[:, fh * 384:(fh + 1) * 384],
                                start=False, stop=True)
                        nc.vector.tensor_scalar(
                            out=vS[:, g, 6 * fh:6 * fh + 6, 0:DH],
                            in0=pm[:, :384], scalar1=DQ_WX, scalar2=None,
                            op0=OP.mult)

                # ---- attention (head pairs share one [128,S] reciprocal;
                #      each pair's normalize tail is deferred one pair so it
                #      overlaps the next pair's matmul/exp front) ----
                def _attn_tail(st):
                    e, ch, zrows, pcxs = st
                    prb2 = ps.tile([P, 512], F32, tag="ps", name="prb",
                                   space="PSUM")
                    nc.tensor.matmul(prb2[:, :S], lhsT=blkA[:],
                                     rhs=zrows[0][:], start=True, stop=False)
                    nc.tensor.matmul(prb2[:, :S], lhsT=blkB[:],
                                     rhs=zrows[1][:], start=False, stop=True)
                    rb2 = tmp.tile([P, S], F32, tag="rb", name="rb")
                    nc.vector.reciprocal(rb2[:], prb2[:, :S])
                    for hh in range(2):
                        r0 = hh * DH
                        nc.vector.tensor_tensor(
                            out=cT[r0:r0 + DH, ch, e * S:(e + 1) * S],
                            in0=pcxs[hh][:DH, :S],
                            in1=rb2[hh * DH:(hh + 1) * DH, :], op=OP.mult)

                prev_st = None
                for e in range(BC):
                    for ch in range(NH // 2):
                        ET2 = tmp.tile([P, 2, 2, S], BF16, tag="ET", name="ET")
                        zrows = [tmp.tile([1, S], BF16, tag=f"zr{i}",
                                          name=f"zr{i}") for i in range(2)]
                        pcxs = []
                        for hh in range(2):
                            r0 = hh * DH
                            psc = ps.tile([P, 512], F32, tag="ps", name="psc",
                                          space="PSUM")
                            for kt in range(2):
                                nc.tensor.matmul(
                                    psc[:, kt * S:(kt + 1) * S],
                                    lhsT=kT[r0:r0 + DH, ch,
                                            e * S + kt * P:
                                            e * S + (kt + 1) * P],
                                    rhs=qT[r0:r0 + DH, ch, e * S:(e + 1) * S],
                                    start=True, stop=True)
                            if mask_ones:
                                # one fused exp over both key tiles
                                nc.scalar.activation(
                                    ET2[:, hh].rearrange("p k s -> p (k s)"),
                                    psc[:], AF.Exp, bias=0.0, scale=0.125)
                            else:
                                for kt in range(2):
                                    nc.scalar.activation(
                                        ET2[:, hh, kt, :],
                                        psc[:, kt * S:(kt + 1) * S],
                                        AF.Exp, bias=mcol[:, kt, e:e + 1],
                                        scale=0.125)
                            h = 2 * ch + hh
                            pcx = ps.tile([P, 512], F32, tag="ps", name="pcx",
                                          space="PSUM")
                            for kt in range(2):
                                nc.tensor.matmul(
                                    pcx[:DH + 1, :S],
                                    lhsT=vS[:, 2 * e + kt, h, :],
                                    rhs=ET2[:, hh, kt, :],
                                    start=(kt == 0), stop=(kt == 1))
                            nc.scalar.copy(zrows[hh][:], pcx[DH:DH + 1, :S])
                            pcxs.append(pcx)
                        if prev_st is not None:
                            _attn_tail(prev_st)
                        prev_st = (e, ch, zrows, pcxs)
                _attn_tail(prev_st)

                # ---- output proj + residual + LN1 ----
                for g in range(NT):
                    xf = tmp.tile([P, H], F32, tag="xf", name="xf")
                    s1a = tmp.tile([P, 1], F32, tag="s1a", name="s1a")
                    s1b = tmp.tile([P, 1], F32, tag="s1b", name="s1b")
                    for fh in range(2):
                        pm = ps.tile([P, 512], F32, tag="ps", name="po",
                                     space="PSUM")
                        for c2 in range(3):
                            nc.tensor.matmul(
                                pm[:, :384],
                                lhsT=cT[:, 2 * c2:2 * c2 + 2,
                                        g * P:(g + 1) * P],
                                rhs=wo[:, c2, :, fh * 384:(fh + 1) * 384],
                                start=(c2 == 0),
                                stop=(c2 == 2 and zero_bias),
                                perf_mode=DR)
                        if not zero_bias:
                            nc.tensor.matmul(
                                pm[:, :384], lhsT=ones_bf[:, :P],
                                rhs=borow[:, fh * 384:(fh + 1) * 384],
                                start=False, stop=True)
                        if os.environ.get("BERTCRF_OLDLN"):
                            nc.vector.scalar_tensor_tensor(
                                out=xf[:, fh * 384:(fh + 1) * 384],
                                in0=pm[:, :384], scalar=DQ_WX,
                                in1=x_sb[:, g, fh * 384:(fh + 1) * 384],
                                op0=OP.mult, op1=OP.add, accum_out=None)
                        else:
                            nc.vector.scalar_tensor_tensor(
                                out=xf[:, fh * 384:(fh + 1) * 384],
                                in0=pm[:, :384], scalar=DQ_WX,
                                in1=x_sb[:, g, fh * 384:(fh + 1) * 384],
                                op0=OP.mult, op1=OP.add,
                                accum_out=(s1a[:] if fh == 0 else s1b[:]))
                    if os.environ.get("BERTCRF_OLDLN"):
                        s1g = None
                    else:
                        s1g = tmp.tile([P, 1], F32, tag="s1g", name="s1g",
                                       bufs=4)
                        nc.vector.tensor_tensor(out=s1g[:], in0=s1a[:],
                                                in1=s1b[:], op=OP.add)
                    _layernorm(nc, tmp, x_sb[:, g, :], xf,
                               None if unit_ln else (G1, B1t), s1=s1g)

                # ---- FFN (two token-half passes) ----
                for th in range(2):
                    tiles = list(range(4 * th, 4 * th + 4))
                    _transpose_tiles(nc, ps, xT, x_sb, ident, tiles)
                    for j in range(FC):
                        w1j = wts.tile([P, 3, 2, P], FP8, tag="w1j",
                                       name="w1j", bufs=4)
                        nc.sync.dma_start(w1j[:], w1_d[l, j])
                        pm = ps.tile([P, 512], F32, tag="ps", name="ph",
                                     space="PSUM")
                        for c2 in range(3):
                            nc.tensor.matmul(
                                pm[:], lhsT=w1j[:, c2],
                                rhs=xT[:, 2 * c2:2 * c2 + 2,
                                       th * 512:(th + 1) * 512],
                                start=(c2 == 0), stop=(c2 == 2),
                                perf_mode=DR)
                        bias = 0.0 if zero_bias else b1_sb[:, j:j + 1]
                        gelu_f = AF.Identity if os.environ.get(
                            "BERTCRF_SIMGELU") else AF.Gelu
                        nc.scalar.activation(hT[:, j, :], pm[:], gelu_f,
                                             bias=bias, scale=DQ_WX)
                    # FFN2: f-half outer so W2 streams once per (th, fh)
                    xfs = [tmp.tile([P, H], F32, tag="xff", name="xff", bufs=4)
                           for _ in range(4)]
                    s1as = [tmp.tile([P, 1], F32, tag="s1fa", name="s1fa",
                                     bufs=4) for _ in range(4)]
                    s1bs = [tmp.tile([P, 1], F32, tag="s1fb", name="s1fb",
                                     bufs=4) for _ in range(4)]
                    for fh in range(2):
                        pms = [ps.tile([P, 512], F32, tag="ps", name="pf2",
                                       space="PSUM") for _ in range(4)]
                        for c2 in range(FC // 2):
                            w2c = wts.tile([P, 2, 384], FP8, tag="w2c",
                                           name="w2c", bufs=6)
                            nc.sync.dma_start(
                                w2c[:],
                                w2_d[l, c2, :, :, fh * 384:(fh + 1) * 384])
                            for gi in range(4):
                                nc.tensor.matmul(
                                    pms[gi][:, :384],
                                    lhsT=hT[:, 2 * c2:2 * c2 + 2,
                                            gi * P:(gi + 1) * P],
                                    rhs=w2c[:],
                                    start=(c2 == 0),
                                    stop=(c2 == FC // 2 - 1 and zero_bias),
                                    perf_mode=DR)
                        if not zero_bias:
                            for gi in range(4):
                                nc.tensor.matmul(
                                    pms[gi][:, :384], lhsT=ones_bf[:, :P],
                                    rhs=b2row[:, fh * 384:(fh + 1) * 384],
                                    start=False, stop=True)
                        for gi in range(4):
                            g = tiles[gi]
                            if os.environ.get("BERTCRF_OLDLN"):
                                nc.vector.scalar_tensor_tensor(
                                    out=xfs[gi][:, fh * 384:(fh + 1) * 384],
                                    in0=pms[gi][:, :384], scalar=DQ_W,
                                    in1=x_sb[:, g, fh * 384:(fh + 1) * 384],
                                    op0=OP.mult, op1=OP.add, accum_out=None)
                            else:
                                nc.vector.scalar_tensor_tensor(
                                    out=xfs[gi][:, fh * 384:(fh + 1) * 384],
                                    in0=pms[gi][:, :384], scalar=DQ_W,
                                    in1=x_sb[:, g, fh * 384:(fh + 1) * 384],
                                    op0=OP.mult, op1=OP.add,
                                    accum_out=(s1as[gi][:] if fh == 0
                                               else s1bs[gi][:]))
                    for gi in range(4):
                        if os.environ.get("BERTCRF_OLDLN"):
                            s1g = None
                        else:
                            s1g = tmp.tile([P, 1], F32, tag="s1g", name="s1g",
                                           bufs=4)
                            nc.vector.tensor_tensor(out=s1g[:], in0=s1as[gi][:],
                                                    in1=s1bs[gi][:], op=OP.add)
                        _layernorm(nc, tmp, x_sb[:, tiles[gi], :], xfs[gi],
                                   None if unit_ln else (G2, B2t), s1=s1g)

            if debug:
                nc.sync.dma_start(dd["dbg_x"][:], x_sb[:])

            # =========== classifier ===========
            _PHASE = os.environ.get("BERTCRF_PHASE", "")
            if _PHASE == "emb":
                emT_zero = True
                nc.vector.memset(emT[:], 0.0)
                nc.vector.memset(expEm[:], 1.0)
            clsw = cst.tile([P, 3, 2, 16], FP8, tag="clsw", name="clsw")
            nc.sync.dma_start(clsw[:], clsw_d[:])
            clsb = cst.tile([K, 1], F32, tag="clsb", name="clsb")
            nc.sync.dma_start(clsb[:], clsb_d[:])
            _transpose_tiles(nc, ps, xT, x_sb, ident, range(NT))
            for t2 in (() if _PHASE == "emb" else range(2)):
                pm = ps.tile([P, 512], F32, tag="ps", name="pcls", space="PSUM")
                for c2 in range(3):
                    nc.tensor.matmul(
                        pm[:K, :], lhsT=clsw[:, c2, :, 0:K],
                        rhs=xT[:, 2 * c2:2 * c2 + 2, t2 * 512:(t2 + 1) * 512],
                        start=(c2 == 0), stop=(c2 == 2), perf_mode=DR)
                nc.scalar.activation(emT[:, t2 * 512:(t2 + 1) * 512],
                                     pm[:K, :], AF.Identity, bias=clsb[:, :1],
                                     scale=DQ_WX)
            if _PHASE != "emb":
                nc.scalar.activation(expEm[:], emT[:], AF.Exp)
            if debug:
                nc.sync.dma_start(dd["dbg_em"][:], emT[:])

        # =========== CRF (weights/tmp pools closed; SBUF freed) ===========
        with contextlib.ExitStack() as cctx:
            crf = cctx.enter_context(tc.tile_pool(name="crf", bufs=1))
            ctmp = cctx.enter_context(tc.tile_pool(name="ctmp", bufs=4))

            def ct(name, shape, dtype=F32):
                return crf.tile(shape, dtype, tag=name, name=name)

            transm = ct("transm", [K, K])
            nc.sync.dma_start(transm[:], transm_d[:])
            Mexp = ct("Mexp", [K, K])
            nc.scalar.activation(Mexp[:], transm[:], AF.Exp)
            startc = ct("startc", [K, 1])
            endc = ct("endc", [K, 1])
            nc.sync.dma_start(startc[:], startc_d[:])
            nc.sync.dma_start(endc[:], endc_d[:])
            expStart = ct("expStart", [K, 1])
            expEnd = ct("expEnd", [K, 1])
            nc.scalar.activation(expStart[:], startc[:], AF.Exp)
            nc.scalar.activation(expEnd[:], endc[:], AF.Exp)
            i9 = ct("i9", [K, 1])
            i81 = ct("i81", [81, 1])
            nc.sync.dma_start(i9[:], i9_d[:])
            nc.sync.dma_start(i81[:], i81_d[:])
            transcol = ct("transcol", [81, 1])
            nc.sync.dma_start(transcol[:], transcol_d[:])

            lrow_i = ct("lrow_i", [1, T], I32)
            nc.sync.dma_start(lrow_i[:], labels_d[:])
            lrow = ct("lrow", [1, T])
            nc.vector.tensor_copy(lrow[:], lrow_i[:])
            mrow_i = ct("mrow_i", [1, T], I32)
            nc.sync.dma_start(mrow_i[:], maskrow_d[:])
            mrow = ct("mrow", [1, T])
            nc.vector.tensor_copy(mrow[:], mrow_i[:])
            wrow = ct("wrow", [1, T])
            nc.vector.tensor_copy(wrow[:], mrow[:])
            nc.vector.memset(
                wrow[:].rearrange("o (b s) -> o b s", b=BC)[:, :, 0:1], 1.0)
            ilrow = ct("ilrow", [1, T])
            m3 = mrow[:].rearrange("o (b s) -> o b s", b=BC)
            il3 = ilrow[:].rearrange("o (b s) -> o b s", b=BC)
            nc.vector.tensor_tensor(out=il3[:, :, 0:S - 1],
                                    in0=m3[:, :, 0:S - 1],
                                    in1=m3[:, :, 1:S], op=OP.subtract)
            nc.vector.tensor_copy(il3[:, :, S - 1:S], m3[:, :, S - 1:S])

            prow = ct("prow", [1, T])
            nc.vector.memset(prow[:, T - 1:T], 0.0)
            nc.vector.tensor_scalar(out=prow[:, 0:T - 1], in0=lrow[:, 0:T - 1],
                                    scalar1=9.0, scalar2=None, op0=OP.mult)
            nc.vector.tensor_tensor(out=prow[:, 0:T - 1], in0=prow[:, 0:T - 1],
                                    in1=lrow[:, 1:T], op=OP.add)

            if _PHASE in ("emb", "cls"):
                skip9 = ct("skip_out", [1, 4])
                nc.vector.memset(skip9[:], 0.0)
                nc.sync.dma_start(out_d[:], skip9[:])
            else:
                lb9 = ct("lb9", [K, T])
                _outer_f32(nc, ps, lb9, ones_f[:, :K], lrow)
                OH9 = ct("OH9", [K, T])
                nc.vector.tensor_scalar(out=OH9[:], in0=lb9[:], scalar1=i9[:, :1],
                                        scalar2=None, op0=OP.is_equal)
                pb81 = ct("pb81", [81, T])
                _outer_f32(nc, ps, pb81, ones_f[:, :81], prow)
                OH81 = ct("OH81", [81, T])
                nc.vector.tensor_scalar(out=OH81[:, 0:T - 1],
                                        in0=pb81[:, 0:T - 1],
                                        scalar1=i81[:, :1], scalar2=None,
                                        op0=OP.is_equal)
                w9 = ct("w9", [K, T])
                _outer_f32(nc, ps, w9, ones_f[:, :K], wrow)
                w81 = ct("w81", [81, T])
                _outer_f32(nc, ps, w81, ones_f[:, :81], mrow)
                il9 = ct("il9", [K, T])
                _outer_f32(nc, ps, il9, ones_f[:, :K], ilrow)

                acc = ct("acc", [P, 4])
                nc.vector.memset(acc[:], 0.0)
                oh9w = ct("oh9w", [K, T])
                nc.vector.tensor_tensor(out=oh9w[:], in0=OH9[:], in1=w9[:],
                                        op=OP.mult)
                sink9 = ct("sink9", [K, T])
                nc.vector.scalar_tensor_tensor(
                    out=sink9[:], in0=emT[:], scalar=1.0, in1=oh9w[:],
                    op0=OP.mult, op1=OP.mult, accum_out=acc[:K, 0:1])
                sink81 = ct("sink81", [81, T])
                nc.vector.scalar_tensor_tensor(
                    out=sink81[:, 0:T - 1], in0=OH81[:, 0:T - 1],
                    scalar=transcol[:, :1], in1=w81[:, 1:T], op0=OP.mult,
                    op1=OP.mult, accum_out=acc[:81, 1:2])
                oh9r = OH9[:].rearrange("k (b s) -> k b s", b=BC)
                sgt = ctmp.tile([K, BC], F32, tag="sgt", name="sgt")
                nc.vector.tensor_scalar(out=sgt[:], in0=oh9r[:, :, 0],
                                        scalar1=startc[:, :1], scalar2=None,
                                        op0=OP.mult, op1=OP.add,
                                        accum_out=acc[:K, 2:3])
                sink9b = ct("sink9b", [K, T])
                nc.vector.scalar_tensor_tensor(
                    out=sink9b[:], in0=OH9[:], scalar=endc[:, :1], in1=il9[:],
                    op0=OP.mult, op1=OP.mult, accum_out=acc[:K, 3:4])
                if debug:
                    nc.sync.dma_start(dd["dbg_acc"][:], acc[:])

                if _PHASE == "num":
                    skip8 = ct("skip_out2", [1, 4])
                    nc.vector.memset(skip8[:], 0.0)
                    nc.sync.dma_start(out_d[:], skip8[:])
                    return
                # ---- linear-space scan ----
                if not mask_ones:
                    inv9 = ct("inv9", [K, T])
                    mb9 = ct("mb9", [K, T])
                    _outer_f32(nc, ps, mb9, ones_f[:, :K], mrow)
                    nc.vector.tensor_scalar(out=inv9[:], in0=mb9[:], scalar1=0.0,
                                            scalar2=None, op0=OP.is_equal)
                    inv4 = inv9[:].rearrange("k (b s) -> k b s", b=BC)

                expEm4 = expEm[:].rearrange("k (b s) -> k b s", b=BC)
                Ecur = ctmp.tile([K, BC], F32, tag="E", name="E0")
                nc.vector.tensor_scalar(out=Ecur[:], in0=expEm4[:, :, 0],
                                        scalar1=expStart[:, :1], scalar2=None,
                                        op0=OP.mult)
                for t in range(1, S):
                    psn = ps.tile([P, 512], F32, tag="ps", name="pcrf",
                                  space="PSUM")
                    nc.tensor.matmul(psn[:K, :BC], lhsT=Mexp[:], rhs=Ecur[:],
                                     start=True, stop=True)
                    Enew = ctmp.tile([K, BC], F32, tag="E", name=f"E{t}")
                    nc.vector.tensor_tensor(out=Enew[:], in0=psn[:K, :BC],
                                            in1=expEm4[:, :, t], op=OP.mult)
                    if not mask_ones:
                        nc.vector.copy_predicated(Enew[:], inv4[:, :, t], Ecur[:])
                    Ecur = Enew
                if debug:
                    nc.sync.dma_start(dd["dbg_ef"][:], Ecur[:])

                F_ = ctmp.tile([K, BC], F32, tag="F", name="F_")
                nc.vector.tensor_scalar(out=F_[:], in0=Ecur[:],
                                        scalar1=expEnd[:, :1], scalar2=None,
                                        op0=OP.mult)
                psd = ps.tile([P, 512], F32, tag="ps", name="psd", space="PSUM")
                nc.tensor.matmul(psd[:1, :BC], lhsT=onescol_f[:K, :], rhs=F_[:],
                                 start=True, stop=True)
                denomv = ctmp.tile([1, BC], F32, tag="denomv", name="denomv")
                denom_tot = ct("denom_tot", [1, 1])
                nc.scalar.activation(denomv[:], psd[:1, :BC], AF.Ln,
                                     accum_out=denom_tot[:])

                psn2 = ps.tile([P, 512], F32, tag="ps", name="psn2", space="PSUM")
                nc.tensor.matmul(psn2[:1, :4], lhsT=onescol_f[:, :1], rhs=acc[:],
                                 start=True, stop=True)
                num_tot = ct("num_tot", [1, 1])
                nc.vector.tensor_reduce(out=num_tot[:], in_=psn2[:1, :4],
                                        axis=AX.X, op=OP.add)
                out_sb = ct("out_sb", [1, 4])
                nc.vector.memset(out_sb[:], 0.0)
                nc.vector.tensor_tensor(out=out_sb[:, 0:1], in0=denom_tot[:],
                                        in1=num_tot[:], op=OP.subtract)
                nc.vector.tensor_copy(out_sb[:, 1:2], num_tot[:])
                nc.vector.tensor_copy(out_sb[:, 2:3], denom_tot[:])
                nc.sync.dma_start(out_d[:], out_sb[:])


# ---------------------------------------------------------------------------
# host wrapper
# ---------------------------------------------------------------------------

_NC_CACHE = {}


def _get_nc(key):
    if key not in _NC_CACHE:
        _NC_CACHE[key] = build_nc(*key)
    return _NC_CACHE[key]


def prepare_maps(inputs, mask_ones, zero_bias, unit_ln):
    input_ids = np.asarray(inputs["input_ids"]).astype(np.int32)
    attention_mask = np.asarray(inputs["attention_mask"]).astype(np.int32)
    labels = np.asarray(inputs["labels"]).astype(np.int32)

    word = _bf(inputs["word_emb"])
    pt = _bf((_f32(inputs["pos_emb"][:S]) + _f32(inputs["type_emb"][0])[None, :])
             .reshape(2, P, H).transpose(1, 0, 2))
    wq = _f8(inputs["Wq"], WS).reshape(L, 3, 2, P, H).transpose(
        0, 3, 1, 2, 4).copy()
    wk = _f8(inputs["Wk"], WS).reshape(L, 3, 2, P, H).transpose(
        0, 3, 1, 2, 4).copy()
    wv = _f8(inputs["Wv"], WS).reshape(L, 3, 2, P, H).transpose(
        0, 3, 1, 2, 4).copy()
    wo = _f8(inputs["Wo"], WS).reshape(L, 3, 2, P, H).transpose(
        0, 3, 1, 2, 4).copy()
    w1 = (_f8(inputs["W1"], WS).reshape(L, 3, 2, P, FC, P)
          .transpose(0, 4, 3, 1, 2, 5).copy())
    w2 = (_f8(inputs["W2"], WS).reshape(L, FC // 2, 2, P, H)
          .transpose(0, 1, 3, 2, 4).copy())
    cwpad = np.zeros((H, 16), np.float32)
    cwpad[:, :K] = _f32(inputs["cls_W"])
    clsw = _f8(cwpad, WS).reshape(3, 2, P, 16).transpose(2, 0, 1, 3).copy()
    clsb = (_f32(inputs["cls_b"]) - np.float32(C_SHIFT)).reshape(K, 1)

    shared = dict(
        word=word, pt=pt, wq=wq, wk=wk, wv=wv, wo=wo, w1=w1, w2=w2,
        clsw=clsw, clsb=clsb,
        transm=_f32(inputs["crf_trans"]).reshape(K, K),
        transcol=_f32(inputs["crf_trans"]).reshape(81, 1),
        startc=_f32(inputs["crf_start"]).reshape(K, 1),
        endc=_f32(inputs["crf_end"]).reshape(K, 1),
        i9=np.arange(K, dtype=np.float32).reshape(K, 1),
        i81=np.arange(81, dtype=np.float32).reshape(81, 1),
        blk2=_bf(((np.arange(P)[None, :] // 64) == np.arange(2)[:, None])
         .astype(np.float32) / XS),
    )
    if not zero_bias:
        shared.update(
            bq=_f32(inputs["bq"]).reshape(L, HC, P).transpose(0, 2, 1).copy(),
            bk=_f32(inputs["bk"]).reshape(L, HC, P).transpose(0, 2, 1).copy(),
            b1=_f32(inputs["b1"]).reshape(L, FC, P).transpose(0, 2, 1).copy(),
            bvrow=_bf(_f32(inputs["bv"]) / DQ_WX).reshape(L, 1, H),
            borow=_bf(_f32(inputs["bo"]) / DQ_WX).reshape(L, 1, H),
            b2row=_bf(_f32(inputs["b2"]) / DQ_W).reshape(L, 1, H),
        )
    if not unit_ln:
        shared.update(
            lng=np.stack([_bf(inputs["ln1_g"]), _bf(inputs["ln2_g"])],
                         axis=1).reshape(L, 2, 1, H),
            lnb=np.stack([_bf(inputs["ln1_b"]), _bf(inputs["ln2_b"])],
                         axis=1).reshape(L, 2, 1, H),
            elng=np.stack([_bf(inputs["emb_ln_g"]), _bf(inputs["emb_ln_b"])],
                          axis=0).reshape(2, 1, H),
        )

    in_maps = []
    for c in range(CORES):
        ids_c = input_ids[BC * c:BC * (c + 1)].reshape(NT, P, 1).copy()
        lab_c = labels[BC * c:BC * (c + 1)].reshape(1, T).copy()
        msk_c = attention_mask[BC * c:BC * (c + 1)]
        m = dict(shared)
        m["ids"] = ids_c
        m["labels"] = lab_c
        m["maskrow"] = msk_c.reshape(1, T).copy()
        if not mask_ones:
            m["maskcols"] = (msk_c.reshape(BC, 2, P).transpose(2, 1, 0)
                             .astype(np.int32).copy())
        in_maps.append(m)
    return in_maps


def kernel(**inputs) -> np.ndarray:
    attention_mask = np.asarray(inputs["attention_mask"])
    assert np.asarray(inputs["input_ids"]).shape == (B, S)

    mask_ones = bool(np.all(attention_mask == 1))
    zero_bias = all(
        not np.any(np.asarray(inputs[k]))
        for k in ("bq", "bk", "bv", "bo", "b1", "b2"))
    unit_ln = (all(np.all(np.asarray(inputs[k]) == 1.0)
                   for k in ("emb_ln_g", "ln1_g", "ln2_g"))
               and all(not np.any(np.asarray(inputs[k]))
                       for k in ("emb_ln_b", "ln1_b", "ln2_b")))

    n_layers = int(os.environ.get("BERTCRF_LAYERS", L))
    debug = bool(int(os.environ.get("BERTCRF_DEBUG", "0")))
    nc = _get_nc((n_layers, mask_ones, zero_bias, unit_ln, debug))
    in_maps = prepare_maps(inputs, mask_ones, zero_bias, unit_ln)

    res = run_bass_kernel_spmd(nc, in_maps, core_ids=list(range(CORES)))
    total = np.float32(0.0)
    for c in range(CORES):
        total += np.float32(res.results[c]["out"][0, 0])
    return np.float32(total)


if __name__ == "__main__":
    import jax
    jax.config.update("jax_platforms", "cpu")
    import reference
    inp = reference.setup_inputs()
    outv = kernel(**{k: np.asarray(v) for k, v in inp.items()})
    print("kernel:", outv)

